# revision 17
# baseline (speedup 1.0000x reference)
"""Causal self-attention block (QKV proj + causal MHA + out proj + residual
+ LayerNorm) for B=4, S=2048, HID=1024, 16 heads, on 8 Trainium2 cores.

Sharding: core c handles batch b=c//2 and heads [8h, 8h+8) where h=c%2
(Megatron-style head split within a batch pair). Each core computes its 8
heads' attention and a partial output projection over the full 2048 rows;
the two cores of a batch pair combine partials with pairwise
ReduceScatters (chunked, pipelined with compute), then each core applies
residual + LayerNorm to its quarter-rows and returns [1024, 1024].

All matmuls run in float32r (TF32-like, ~11 mantissa bits, 1 cycle/row at
moving-dim>=256). Attention uses the transposed-score layout
(scoresT[sk, sq]): softmax sums fall out of the PV matmul via an appended
ones-row on V, causal structure shrinks above-diagonal tiles, and each
head pair shares fused two-bank PSUM tiles so one ACT exp covers both
heads. ScalarE runs exp only; all PSUM evacuation goes through VectorE.
"""

import numpy as np

import concourse.bacc as bacc
import concourse.mybir as mybir
import concourse.tile as tile
from concourse.bass_utils import run_bass_kernel_spmd

F32 = mybir.dt.float32
F32R = mybir.dt.float32r
AF = mybir.ActivationFunctionType
OP = mybir.AluOpType

N_CORES = 8
B, S, HID = 4, 2048, 1024
NHC = 8          # heads per core
DH = 64          # head dim
HW = 512         # per-core head width (NHC * DH)
SQT = 512        # sq tile width
NSQT = S // SQT  # 4
NHCH = HID // 128  # 8 hid chunks
SH = S // 2      # rows per core in the epilogue
EPS = 1e-5

_CACHE = {}


def _build():
    nc = bacc.Bacc("TRN2", target_bir_lowering=False, debug=False,
                   num_devices=N_CORES)

    xT = nc.dram_tensor("xT", [HID, S], F32R, kind="ExternalInput").ap()
    xh = nc.dram_tensor("xh", [SH, HID], F32, kind="ExternalInput").ap()
    wqT = nc.dram_tensor("wqT", [HID, HW], F32R, kind="ExternalInput").ap()
    wkT = nc.dram_tensor("wkT", [HID, HW], F32R, kind="ExternalInput").ap()
    wvT = nc.dram_tensor("wvT", [HID, HW], F32R, kind="ExternalInput").ap()
    woT = nc.dram_tensor("woT", [HW, HID], F32R, kind="ExternalInput").ap()
    bq4 = nc.dram_tensor("bq4", [128, 4], F32, kind="ExternalInput").ap()
    bk4 = nc.dram_tensor("bk4", [128, 4], F32, kind="ExternalInput").ap()
    bvb = nc.dram_tensor("bvb", [128, HW], F32, kind="ExternalInput").ap()
    gmb = nc.dram_tensor("gmb", [128, HID], F32, kind="ExternalInput").ap()
    btb = nc.dram_tensor("btb", [128, HID], F32, kind="ExternalInput").ap()
    m128 = nc.dram_tensor("m128", [128, 128], F32, kind="ExternalInput").ap()
    vone = nc.dram_tensor("vone", [128, 8], F32R, kind="ExternalInput").ap()
    one64 = nc.dram_tensor("one64", [1, 64], F32R, kind="ExternalInput").ap()

    out = nc.dram_tensor("out", [SH, HID], F32, kind="ExternalOutput").ap()

    po_d = nc.dram_tensor("po_d", [S, HID], F32)
    rs_d = nc.dram_tensor("rs_d", [SH, HID], F32)

    from contextlib import ExitStack
    with tile.TileContext(nc) as tc, ExitStack() as es:
        TP = tc.tile_pool
        cp = es.enter_context(TP(name="consts", bufs=1))
        ktp = es.enter_context(TP(name="kt", bufs=1))
        vtp = es.enter_context(TP(name="vt", bufs=1))
        wop = es.enter_context(TP(name="wo", bufs=1))
        ep = es.enter_context(TP(name="exp", bufs=2))
        atp = es.enter_context(TP(name="att", bufs=1))
        avp = es.enter_context(TP(name="av", bufs=2))
        rp = es.enter_context(TP(name="rcp", bufs=2))
        poep = es.enter_context(TP(name="poe", bufs=1))
        pp = es.enter_context(TP(name="pp", bufs=2, space="PSUM"))
        sp = es.enter_context(TP(name="sp", bufs=2, space="PSUM"))
        app = es.enter_context(TP(name="ap", bufs=1, space="PSUM"))
        if True:

            # ---- constants ----
            mask = cp.tile([128, 128], F32)
            nc.sync.dma_start(mask[:], m128[:])
            bqs = cp.tile([128, 4], F32)
            nc.sync.dma_start(bqs[:], bq4[:])
            bks = cp.tile([128, 4], F32)
            nc.sync.dma_start(bks[:], bk4[:])
            bvs = cp.tile([128, HW], F32)
            nc.sync.dma_start(bvs[:], bvb[:])
            vos = cp.tile([128, 8], F32R)
            nc.sync.dma_start(vos[:], vone[:])
            epsc = cp.tile([128, 1], F32)
            nc.vector.memset(epsc[:], EPS)
            o64 = cp.tile([1, 64], F32R)
            nc.sync.dma_start(o64[:], one64[:])
            gms = cp.tile([128, HID], F32)
            nc.sync.dma_start(gms[:], gmb[:])
            bts = cp.tile([128, HID], F32)
            nc.sync.dma_start(bts[:], btb[:])

            # ---- weights resident in SBUF ----
            wot = []
            for d in range(4):
                w = wop.tile([128, HID], F32R, name=f"wo{d}")
                nc.sync.dma_start(w[:], woT[128 * d:128 * (d + 1), :])
                wot.append(w)

            kt = [ktp.tile([128, S], F32R, name=f"kt{p}") for p in range(4)]
            vt = [vtp.tile([128, 8, 65], F32R, name=f"vt{i}")
                  for i in range(16)]

            wp = es.enter_context(TP(name="wqkv", bufs=1))
            xp = es.enter_context(TP(name="xts", bufs=1))
            qtp = es.enter_context(TP(name="qt", bufs=1))
            lp = es.enter_context(TP(name="ln", bufs=1))
            lsp = es.enter_context(TP(name="lns", bufs=2))
            if True:

                wq, wk, wv = [], [], []
                for nm, dr, lst in (("wq", wqT, wq), ("wk", wkT, wk),
                                    ("wv", wvT, wv)):
                    for hh in range(NHCH):
                        w = wp.tile([128, HW], F32R, name=f"{nm}{hh}")
                        nc.sync.dma_start(w[:], dr[128 * hh:128 * (hh + 1), :])
                        lst.append(w)

                def emit_ln(c_):
                    """Residual + LayerNorm for output chunk c_ (128 rows)."""
                    rs = lp.tile([128, HID], F32, tag="rs")
                    nc.sync.dma_start(rs[:], rs_d[128 * c_:128 * (c_ + 1), :])
                    xc = lp.tile([128, HID], F32, tag="xc")
                    nc.sync.dma_start(xc[:], xh[128 * c_:128 * (c_ + 1), :])
                    nc.vector.tensor_add(rs[:], rs[:], xc[:])
                    st6 = lsp.tile([128, 12], F32, tag="st6")
                    nc.vector.bn_stats(st6[:, 0:6], rs[:, 0:512])
                    nc.vector.bn_stats(st6[:, 6:12], rs[:, 512:1024])
                    mv = lsp.tile([128, 2], F32, tag="mv")
                    nc.vector.bn_aggr(mv[:], st6[:])
                    sd = lsp.tile([128, 1], F32, tag="sd")
                    nc.scalar.activation(sd[:], mv[:, 1:2], AF.Sqrt,
                                         bias=epsc[:])
                    inv = lsp.tile([128, 1], F32, tag="inv")
                    nc.vector.reciprocal_approx_fast(inv[:], sd[:])
                    nc.vector.tensor_scalar(xc[:], rs[:], mv[:, 0:1], inv[:],
                                            op0=OP.subtract, op1=OP.mult)
                    nc.vector.tensor_mul(rs[:], xc[:], gms[:])
                    nc.vector.tensor_add(rs[:], rs[:], bts[:])
                    nc.sync.dma_start(out[128 * c_:128 * (c_ + 1), :], rs[:])

                for t in range(NSQT):
                    # ---- delayed LayerNorm for the previous tile ----
                    if t > 0:
                        emit_ln(2 * (t - 1))
                        emit_ln(2 * (t - 1) + 1)
                    # ---- phase A: projections for sq tile t ----
                    xts = []
                    for hh in range(NHCH):
                        xt_ = xp.tile([128, SQT], F32R, tag=f"xt{hh}")
                        nc.sync.dma_start(
                            xt_[:], xT[128 * hh:128 * (hh + 1),
                                       SQT * t:SQT * (t + 1)])
                        xts.append(xt_)

                    qts = []
                    for m in range(4):
                        ps = pp.tile([128, SQT], F32, tag="pq")
                        for hh in range(NHCH):
                            nc.tensor.matmul(
                                ps[:], wq[hh][:, 128 * m:128 * (m + 1)],
                                xts[hh][:], start=(hh == 0),
                                stop=(hh == NHCH - 1))
                        qt_ = qtp.tile([128, SQT], F32R, tag=f"q{m}")
                        nc.vector.tensor_scalar_add(qt_[:], ps[:],
                                                    bqs[:, m:m + 1])
                        qts.append(qt_)
                    for m in range(4):
                        ps = pp.tile([128, SQT], F32, tag="pq")
                        for hh in range(NHCH):
                            nc.tensor.matmul(
                                ps[:], wk[hh][:, 128 * m:128 * (m + 1)],
                                xts[hh][:], start=(hh == 0),
                                stop=(hh == NHCH - 1))
                        nc.vector.tensor_scalar_add(
                            kt[m][:, SQT * t:SQT * (t + 1)], ps[:],
                            bks[:, m:m + 1])
                    for s_ in range(4):
                        i = 4 * t + s_
                        ps = pp.tile([128, HW], F32, tag="pq")
                        for hh in range(NHCH):
                            nc.tensor.matmul(
                                ps[:], xts[hh][:, 128 * s_:128 * (s_ + 1)],
                                wv[hh][:], start=(hh == 0),
                                stop=(hh == NHCH - 1))
                        nc.vector.tensor_tensor(
                            vt[i][:, :, 0:64], ps[:], bvs[:], op=OP.add)
                        nc.vector.tensor_copy(vt[i][:, :, 64:65], vos[:])

                    # ---- phase B: attention for sq tile j = t ----
                    j = t
                    at_tiles = []
                    for p in range(4):
                        pv2 = app.tile([128, 2 * SQT], F32, tag="pv2")
                        for i in range(4 * j + 4):
                            d = i - 4 * j
                            lo_qk = min(128 * d, 256) if d >= 0 else 0
                            lo = 128 * d if d >= 0 else 0
                            s2 = sp.tile([128, 2 * SQT], F32, tag="s2")
                            nc.tensor.matmul(
                                s2[:, lo_qk:SQT],
                                kt[p][0:64, 128 * i:128 * (i + 1)],
                                qts[p][0:64, lo_qk:SQT],
                                start=True, stop=True, tile_position=(0, 0))
                            nc.tensor.matmul(
                                s2[:, SQT + lo_qk:2 * SQT],
                                kt[p][64:128, 128 * i:128 * (i + 1)],
                                qts[p][64:128, lo_qk:SQT],
                                start=True, stop=True, tile_position=(64, 0))
                            e2 = ep.tile([128, 2 * SQT], F32R, tag="e2")
                            s2v = s2[:].rearrange("p (a b) -> p a b", a=2)
                            e2v = e2[:].rearrange("p (a b) -> p a b", a=2)
                            nc.scalar.activation(e2v[:, :, lo:SQT],
                                                 s2v[:, :, lo:SQT],
                                                 AF.Exp, scale=0.125)
                            if d >= 0:
                                nc.vector.tensor_mul(
                                    e2[:, lo:lo + 128], e2[:, lo:lo + 128],
                                    mask[:])
                                nc.vector.tensor_mul(
                                    e2[:, SQT + lo:SQT + lo + 128],
                                    e2[:, SQT + lo:SQT + lo + 128], mask[:])
                            nc.tensor.matmul(
                                pv2[0:65, lo:SQT],
                                vt[i][:, 2 * p, :], e2[:, lo:SQT],
                                start=(i == 0), stop=(i == 4 * j + 3))
                            nc.tensor.matmul(
                                pv2[0:65, SQT + lo:2 * SQT],
                                vt[i][:, 2 * p + 1, :],
                                e2[:, SQT + lo:2 * SQT],
                                start=(i == 0), stop=(i == 4 * j + 3))
                        # quick PSUM evac, then per-pair normalize
                        av2 = avp.tile([65, 2 * SQT], F32, tag="av")
                        nc.vector.tensor_copy(av2[:], pv2[0:65, :])
                        at_ = atp.tile([128, SQT], F32R, tag=f"at{p}")
                        bc = sp.tile([128, 2 * SQT], F32, tag="s2")
                        for hb in range(2):
                            sm = rp.tile([1, SQT], F32, tag="sm", bufs=1)
                            nc.vector.tensor_copy(
                                sm[:], av2[64:65, SQT * hb:SQT * (hb + 1)])
                            rc = rp.tile([1, SQT], F32, tag="rc", bufs=1)
                            nc.vector.reciprocal_approx_fast(rc[:], sm[:])
                            rcr = rp.tile([1, SQT], F32R, tag="rcr", bufs=1)
                            nc.vector.tensor_copy(rcr[:], rc[:])
                            nc.tensor.matmul(
                                bc[0:64, SQT * hb:SQT * (hb + 1)],
                                o64[:], rcr[:], start=True, stop=True)
                            nc.vector.tensor_mul(
                                at_[64 * hb:64 * (hb + 1), :],
                                av2[0:64, SQT * hb:SQT * (hb + 1)],
                                bc[0:64, SQT * hb:SQT * (hb + 1)])
                        at_tiles.append(at_)

                    # ---- phase C: out projection for sq tile j ----
                    for c_ in range(4):
                        po = poep.tile([128, HID], F32, tag="po")
                        for o in range(2):
                            ps = pp.tile([128, SQT], F32, tag="pq")
                            for dch in range(4):
                                nc.tensor.matmul(
                                    ps[:],
                                    at_tiles[dch][:, 128 * c_:128 * (c_ + 1)],
                                    wot[dch][:, SQT * o:SQT * (o + 1)],
                                    start=(dch == 0), stop=(dch == 3))
                            nc.vector.tensor_copy(
                                po[:, SQT * o:SQT * (o + 1)], ps[:])
                        r0 = SQT * j + 128 * c_
                        nc.sync.dma_start(po_d[r0:r0 + 128, :], po[:])

                    # ---- phase D: pairwise ReduceScatter for this tile ----
                    nc.gpsimd.collective_compute(
                        "ReduceScatter",
                        OP.add,
                        replica_groups=[[0, 1], [2, 3], [4, 5], [6, 7]],
                        ins=[po_d[SQT * j:SQT * (j + 1), :]],
                        outs=[rs_d[256 * j:256 * (j + 1), :]],
                    )

                emit_ln(2 * (NSQT - 1))
                emit_ln(2 * (NSQT - 1) + 1)


    nc.compile()
    return nc


def _prep_inputs(x, Wq, bq, Wk, bk, Wv, bv, Wo, bo, gamma, beta):
    """Shard + lay out the full inputs for the 8 cores."""
    f32 = np.float32
    x = np.asarray(x, f32)
    Wq, bq = np.asarray(Wq, f32), np.asarray(bq, f32)
    Wk, bk = np.asarray(Wk, f32), np.asarray(bk, f32)
    Wv, bv = np.asarray(Wv, f32), np.asarray(bv, f32)
    Wo, bo = np.asarray(Wo, f32), np.asarray(bo, f32)
    gamma, beta = np.asarray(gamma, f32), np.asarray(beta, f32)

    mask = np.triu(np.ones((128, 128), f32))
    vone = np.ones((128, 8), f32)
    gmb = np.ascontiguousarray(np.broadcast_to(gamma, (128, HID)))
    btb = np.ascontiguousarray(np.broadcast_to(beta, (128, HID)))

    halves = []
    for h in range(2):
        sl = slice(HW * h, HW * (h + 1))
        halves.append(dict(
            wqT=np.ascontiguousarray(Wq.T[:, sl]),
            wkT=np.ascontiguousarray(Wk.T[:, sl]),
            wvT=np.ascontiguousarray(Wv.T[:, sl]),
            woT=np.ascontiguousarray(Wo[:, sl].T),
            bq4=np.ascontiguousarray(bq[sl].reshape(4, 128).T),
            bk4=np.ascontiguousarray(bk[sl].reshape(4, 128).T),
            bvb=np.ascontiguousarray(np.broadcast_to(bv[sl], (128, HW))),
        ))

    in_maps = []
    for c in range(N_CORES):
        b, h = c // 2, c % 2
        m = dict(halves[h])
        m["xT"] = np.ascontiguousarray(x[b].T)
        # rows this core receives from the chunked pairwise RS:
        # chunk j covers global rows [512j + 256h, 512j + 256h + 256)
        m["xh"] = np.ascontiguousarray(
            np.concatenate([x[b, 512 * j + 256 * h:512 * j + 256 * h + 256, :]
                            for j in range(4)], axis=0) + bo)
        m["gmb"] = gmb
        m["btb"] = btb
        m["m128"] = mask
        m["vone"] = vone
        m["one64"] = np.ones((1, 64), f32)
        in_maps.append(m)
    return in_maps


def _run(inputs, trace=False):
    if "nc" not in _CACHE:
        _CACHE["nc"] = _build()
    nc = _CACHE["nc"]
    in_maps = _prep_inputs(**inputs)
    res = run_bass_kernel_spmd(nc, in_maps, list(range(N_CORES)),
                               trace=trace)
    out = np.empty((B, S, HID), np.float32)
    for c in range(N_CORES):
        b, h = c // 2, c % 2
        o = res.results[c]["out"]
        for j in range(4):
            out[b, 512 * j + 256 * h:512 * j + 256 * h + 256, :] = \
                o[256 * j:256 * (j + 1), :]
    return out, res


def kernel(**inputs):
    out, _ = _run(inputs, trace=False)
    return out


# revision 18
# speedup vs baseline: 1.1216x; 1.1216x over previous
"""Causal self-attention block (QKV proj + causal MHA + out proj + residual
+ LayerNorm) for B=4, S=2048, HID=1024, 16 heads, on 8 Trainium2 cores.

Sharding: core c handles batch b=c//2 and heads [8h, 8h+8) where h=c%2
(Megatron-style head split within a batch pair). Each core computes its 8
heads' attention and a partial output projection over the full 2048 rows;
the two cores of a batch pair combine partials with pairwise
ReduceScatters (chunked, pipelined with compute), then each core applies
residual + LayerNorm to its quarter-rows and returns [1024, 1024].

All matmuls run in float32r (TF32-like, ~11 mantissa bits, 1 cycle/row at
moving-dim>=256). Attention uses the transposed-score layout
(scoresT[sk, sq]): softmax sums fall out of the PV matmul via an appended
ones-row on V, causal structure shrinks above-diagonal tiles, and each
head pair shares fused two-bank PSUM tiles so one ACT exp covers both
heads. ScalarE runs exp only; all PSUM evacuation goes through VectorE.
"""

import numpy as np

import concourse.bacc as bacc
import concourse.mybir as mybir
import concourse.tile as tile
from concourse.bass_utils import run_bass_kernel_spmd

F32 = mybir.dt.float32
F32R = mybir.dt.float32r
AF = mybir.ActivationFunctionType
OP = mybir.AluOpType

N_CORES = 8
B, S, HID = 4, 2048, 1024
NHC = 8          # heads per core
DH = 64          # head dim
HW = 512         # per-core head width (NHC * DH)
SQT = 512        # sq tile width
NSQT = S // SQT  # 4
NHCH = HID // 128  # 8 hid chunks
SH = S // 2      # rows per core in the epilogue
EPS = 1e-5

_CACHE = {}


def _build():
    nc = bacc.Bacc("TRN2", target_bir_lowering=False, debug=False,
                   num_devices=N_CORES)

    xT = nc.dram_tensor("xT", [HID, S], F32R, kind="ExternalInput").ap()
    xh = nc.dram_tensor("xh", [SH, HID], F32, kind="ExternalInput").ap()
    wqT = nc.dram_tensor("wqT", [HID, HW], F32R, kind="ExternalInput").ap()
    wkT = nc.dram_tensor("wkT", [HID, HW], F32R, kind="ExternalInput").ap()
    wvT = nc.dram_tensor("wvT", [HID, HW], F32R, kind="ExternalInput").ap()
    woT = nc.dram_tensor("woT", [HW, HID], F32R, kind="ExternalInput").ap()
    bq4 = nc.dram_tensor("bq4", [128, 4], F32, kind="ExternalInput").ap()
    bk4 = nc.dram_tensor("bk4", [128, 4], F32, kind="ExternalInput").ap()
    bvb = nc.dram_tensor("bvb", [128, HW], F32, kind="ExternalInput").ap()
    gmb = nc.dram_tensor("gmb", [128, HID], F32, kind="ExternalInput").ap()
    btb = nc.dram_tensor("btb", [128, HID], F32, kind="ExternalInput").ap()
    m128 = nc.dram_tensor("m128", [128, 128], F32, kind="ExternalInput").ap()
    vone = nc.dram_tensor("vone", [128, 8], F32R, kind="ExternalInput").ap()
    one64 = nc.dram_tensor("one64", [1, 64], F32R, kind="ExternalInput").ap()

    out = nc.dram_tensor("out", [SH, HID], F32, kind="ExternalOutput").ap()

    po_d = nc.dram_tensor("po_d", [S, HID], F32)
    rs_d = nc.dram_tensor("rs_d", [SH, HID], F32)

    from contextlib import ExitStack
    with tile.TileContext(nc) as tc, ExitStack() as es:
        TP = tc.tile_pool
        cp = es.enter_context(TP(name="consts", bufs=1))
        ktp = es.enter_context(TP(name="kt", bufs=1))
        vtp = es.enter_context(TP(name="vt", bufs=1))
        wop = es.enter_context(TP(name="wo", bufs=1))
        ep = es.enter_context(TP(name="exp", bufs=2))
        atp = es.enter_context(TP(name="att", bufs=1))
        avp = es.enter_context(TP(name="av", bufs=2))
        rp = es.enter_context(TP(name="rcp", bufs=2))
        poep = es.enter_context(TP(name="poe", bufs=1))
        pp = es.enter_context(TP(name="pp", bufs=2, space="PSUM"))
        sp = es.enter_context(TP(name="sp", bufs=2, space="PSUM"))
        app = es.enter_context(TP(name="ap", bufs=1, space="PSUM"))
        if True:

            # ---- constants ----
            mask = cp.tile([128, 128], F32)
            nc.sync.dma_start(mask[:], m128[:])
            bqs = cp.tile([128, 4], F32)
            nc.sync.dma_start(bqs[:], bq4[:])
            bks = cp.tile([128, 4], F32)
            nc.sync.dma_start(bks[:], bk4[:])
            bvs = cp.tile([128, HW], F32)
            nc.sync.dma_start(bvs[:], bvb[:])
            vos = cp.tile([128, 8], F32R)
            nc.sync.dma_start(vos[:], vone[:])
            epsc = cp.tile([128, 1], F32)
            nc.vector.memset(epsc[:], EPS)
            o64 = cp.tile([1, 64], F32R)
            nc.sync.dma_start(o64[:], one64[:])
            gms = cp.tile([128, HID], F32)
            nc.sync.dma_start(gms[:], gmb[:])
            bts = cp.tile([128, HID], F32)
            nc.sync.dma_start(bts[:], btb[:])

            # ---- weights resident in SBUF ----
            wot = []
            for d in range(4):
                w = wop.tile([128, HID], F32R, name=f"wo{d}")
                nc.sync.dma_start(w[:], woT[128 * d:128 * (d + 1), :])
                wot.append(w)

            kt = [ktp.tile([128, S], F32R, name=f"kt{p}") for p in range(4)]
            vt = [vtp.tile([128, 8, 65], F32R, name=f"vt{i}")
                  for i in range(16)]

            wp = es.enter_context(TP(name="wqkv", bufs=1))
            xp = es.enter_context(TP(name="xts", bufs=1))
            qtp = es.enter_context(TP(name="qt", bufs=1))
            lp = es.enter_context(TP(name="ln", bufs=1))
            lsp = es.enter_context(TP(name="lns", bufs=2))
            if True:

                wq, wk, wv = [], [], []
                for nm, dr, lst in (("wq", wqT, wq), ("wk", wkT, wk),
                                    ("wv", wvT, wv)):
                    for hh in range(NHCH):
                        w = wp.tile([128, HW], F32R, name=f"{nm}{hh}")
                        nc.sync.dma_start(w[:], dr[128 * hh:128 * (hh + 1), :])
                        lst.append(w)

                def emit_ln(c_):
                    """Residual + LayerNorm for output chunk c_ (128 rows)."""
                    rs = lp.tile([128, HID], F32, tag="rs")
                    nc.sync.dma_start(rs[:], rs_d[128 * c_:128 * (c_ + 1), :])
                    xc = lp.tile([128, HID], F32, tag="xc")
                    nc.sync.dma_start(xc[:], xh[128 * c_:128 * (c_ + 1), :])
                    nc.vector.tensor_add(rs[:], rs[:], xc[:])
                    st6 = lsp.tile([128, 12], F32, tag="st6")
                    nc.vector.bn_stats(st6[:, 0:6], rs[:, 0:512])
                    nc.vector.bn_stats(st6[:, 6:12], rs[:, 512:1024])
                    mv = lsp.tile([128, 2], F32, tag="mv")
                    nc.vector.bn_aggr(mv[:], st6[:])
                    sd = lsp.tile([128, 1], F32, tag="sd")
                    nc.scalar.activation(sd[:], mv[:, 1:2], AF.Sqrt,
                                         bias=epsc[:])
                    inv = lsp.tile([128, 1], F32, tag="inv")
                    nc.vector.reciprocal_approx_fast(inv[:], sd[:])
                    nc.vector.tensor_scalar(xc[:], rs[:], mv[:, 0:1], inv[:],
                                            op0=OP.subtract, op1=OP.mult)
                    nc.vector.tensor_mul(rs[:], xc[:], gms[:])
                    nc.vector.tensor_add(rs[:], rs[:], bts[:])
                    nc.sync.dma_start(out[128 * c_:128 * (c_ + 1), :], rs[:])

                for t in range(NSQT):
                    # ---- delayed LayerNorm for the previous tile ----
                    if t > 0:
                        emit_ln(2 * (t - 1))
                        emit_ln(2 * (t - 1) + 1)
                    # ---- phase A: projections for sq tile t ----
                    xts = []
                    for hh in range(NHCH):
                        xt_ = xp.tile([128, SQT], F32R, tag=f"xt{hh}")
                        nc.sync.dma_start(
                            xt_[:], xT[128 * hh:128 * (hh + 1),
                                       SQT * t:SQT * (t + 1)])
                        xts.append(xt_)

                    qts = []
                    for m in range(4):
                        ps = pp.tile([128, SQT], F32, tag="pq")
                        for hh in range(NHCH):
                            nc.tensor.matmul(
                                ps[:], wq[hh][:, 128 * m:128 * (m + 1)],
                                xts[hh][:], start=(hh == 0),
                                stop=(hh == NHCH - 1))
                        qt_ = qtp.tile([128, SQT], F32R, tag=f"q{m}")
                        nc.vector.tensor_scalar_add(qt_[:], ps[:],
                                                    bqs[:, m:m + 1])
                        qts.append(qt_)
                    for m in range(4):
                        ps = pp.tile([128, SQT], F32, tag="pq")
                        for hh in range(NHCH):
                            nc.tensor.matmul(
                                ps[:], wk[hh][:, 128 * m:128 * (m + 1)],
                                xts[hh][:], start=(hh == 0),
                                stop=(hh == NHCH - 1))
                        nc.vector.tensor_scalar_add(
                            kt[m][:, SQT * t:SQT * (t + 1)], ps[:],
                            bks[:, m:m + 1])
                    for s_ in range(4):
                        i = 4 * t + s_
                        ps = pp.tile([128, HW], F32, tag="pq")
                        for hh in range(NHCH):
                            nc.tensor.matmul(
                                ps[:], xts[hh][:, 128 * s_:128 * (s_ + 1)],
                                wv[hh][:], start=(hh == 0),
                                stop=(hh == NHCH - 1))
                        nc.vector.tensor_tensor(
                            vt[i][:, :, 0:64], ps[:], bvs[:], op=OP.add)
                        nc.vector.tensor_copy(vt[i][:, :, 64:65], vos[:])

                    # ---- phase B: attention for sq tile j = t ----
                    j = t
                    at_tiles = []
                    for p in range(4):
                        pv2 = app.tile([128, 2 * SQT], F32, tag="pv2")
                        for i in range(4 * j + 4):
                            d = i - 4 * j
                            lo_qk = min(128 * d, 256) if d >= 0 else 0
                            lo = 128 * d if d >= 0 else 0
                            s2 = sp.tile([128, 2 * SQT], F32, tag="s2")
                            nc.tensor.matmul(
                                s2[:, lo_qk:SQT],
                                kt[p][0:64, 128 * i:128 * (i + 1)],
                                qts[p][0:64, lo_qk:SQT],
                                start=True, stop=True, tile_position=(0, 0))
                            nc.tensor.matmul(
                                s2[:, SQT + lo_qk:2 * SQT],
                                kt[p][64:128, 128 * i:128 * (i + 1)],
                                qts[p][64:128, lo_qk:SQT],
                                start=True, stop=True, tile_position=(64, 0))
                            e2 = ep.tile([128, 2 * SQT], F32R, tag="e2")
                            s2v = s2[:].rearrange("p (a b) -> p a b", a=2)
                            e2v = e2[:].rearrange("p (a b) -> p a b", a=2)
                            nc.scalar.activation(e2v[:, :, lo:SQT],
                                                 s2v[:, :, lo:SQT],
                                                 AF.Exp, scale=0.125)
                            if d >= 0:
                                nc.vector.tensor_mul(
                                    e2[:, lo:lo + 128], e2[:, lo:lo + 128],
                                    mask[:])
                                nc.vector.tensor_mul(
                                    e2[:, SQT + lo:SQT + lo + 128],
                                    e2[:, SQT + lo:SQT + lo + 128], mask[:])
                            nc.tensor.matmul(
                                pv2[0:65, lo:SQT],
                                vt[i][:, 2 * p, :], e2[:, lo:SQT],
                                start=(i == 0), stop=(i == 4 * j + 3))
                            nc.tensor.matmul(
                                pv2[0:65, SQT + lo:2 * SQT],
                                vt[i][:, 2 * p + 1, :],
                                e2[:, SQT + lo:2 * SQT],
                                start=(i == 0), stop=(i == 4 * j + 3))
                        # quick PSUM evac, then per-pair normalize
                        av2 = avp.tile([65, 2 * SQT], F32, tag="av")
                        nc.vector.tensor_copy(av2[:], pv2[0:65, :])
                        at_ = atp.tile([128, SQT], F32R, tag=f"at{p}")
                        for hb in range(2):
                            sm = rp.tile([1, SQT], F32, tag="sm", bufs=1)
                            nc.vector.tensor_copy(
                                sm[:], av2[64:65, SQT * hb:SQT * (hb + 1)])
                            rc = rp.tile([1, SQT], F32, tag="rc", bufs=1)
                            nc.vector.reciprocal_approx_fast(rc[:], sm[:])
                            rb = rp.tile([64, SQT], F32, tag="rb", bufs=1)
                            nc.gpsimd.partition_broadcast(rb[:], rc[:])
                            nc.vector.tensor_mul(
                                at_[64 * hb:64 * (hb + 1), :],
                                av2[0:64, SQT * hb:SQT * (hb + 1)],
                                rb[:])
                        at_tiles.append(at_)

                    # ---- phase C: out projection for sq tile j ----
                    for c_ in range(4):
                        po = poep.tile([128, HID], F32, tag="po")
                        for o in range(2):
                            ps = pp.tile([128, SQT], F32, tag="pq")
                            for dch in range(4):
                                nc.tensor.matmul(
                                    ps[:],
                                    at_tiles[dch][:, 128 * c_:128 * (c_ + 1)],
                                    wot[dch][:, SQT * o:SQT * (o + 1)],
                                    start=(dch == 0), stop=(dch == 3))
                            nc.vector.tensor_copy(
                                po[:, SQT * o:SQT * (o + 1)], ps[:])
                        r0 = SQT * j + 128 * c_
                        nc.sync.dma_start(po_d[r0:r0 + 128, :], po[:])

                    # ---- phase D: pairwise ReduceScatter for this tile ----
                    nc.gpsimd.collective_compute(
                        "ReduceScatter",
                        OP.add,
                        replica_groups=[[0, 1], [2, 3], [4, 5], [6, 7]],
                        ins=[po_d[SQT * j:SQT * (j + 1), :]],
                        outs=[rs_d[256 * j:256 * (j + 1), :]],
                    )

                emit_ln(2 * (NSQT - 1))
                emit_ln(2 * (NSQT - 1) + 1)


    nc.compile()
    return nc


def _prep_inputs(x, Wq, bq, Wk, bk, Wv, bv, Wo, bo, gamma, beta):
    """Shard + lay out the full inputs for the 8 cores."""
    f32 = np.float32
    x = np.asarray(x, f32)
    Wq, bq = np.asarray(Wq, f32), np.asarray(bq, f32)
    Wk, bk = np.asarray(Wk, f32), np.asarray(bk, f32)
    Wv, bv = np.asarray(Wv, f32), np.asarray(bv, f32)
    Wo, bo = np.asarray(Wo, f32), np.asarray(bo, f32)
    gamma, beta = np.asarray(gamma, f32), np.asarray(beta, f32)

    mask = np.triu(np.ones((128, 128), f32))
    vone = np.ones((128, 8), f32)
    gmb = np.ascontiguousarray(np.broadcast_to(gamma, (128, HID)))
    btb = np.ascontiguousarray(np.broadcast_to(beta, (128, HID)))

    halves = []
    for h in range(2):
        sl = slice(HW * h, HW * (h + 1))
        halves.append(dict(
            wqT=np.ascontiguousarray(Wq.T[:, sl]),
            wkT=np.ascontiguousarray(Wk.T[:, sl]),
            wvT=np.ascontiguousarray(Wv.T[:, sl]),
            woT=np.ascontiguousarray(Wo[:, sl].T),
            bq4=np.ascontiguousarray(bq[sl].reshape(4, 128).T),
            bk4=np.ascontiguousarray(bk[sl].reshape(4, 128).T),
            bvb=np.ascontiguousarray(np.broadcast_to(bv[sl], (128, HW))),
        ))

    in_maps = []
    for c in range(N_CORES):
        b, h = c // 2, c % 2
        m = dict(halves[h])
        m["xT"] = np.ascontiguousarray(x[b].T)
        # rows this core receives from the chunked pairwise RS:
        # chunk j covers global rows [512j + 256h, 512j + 256h + 256)
        m["xh"] = np.ascontiguousarray(
            np.concatenate([x[b, 512 * j + 256 * h:512 * j + 256 * h + 256, :]
                            for j in range(4)], axis=0) + bo)
        m["gmb"] = gmb
        m["btb"] = btb
        m["m128"] = mask
        m["vone"] = vone
        m["one64"] = np.ones((1, 64), f32)
        in_maps.append(m)
    return in_maps


def _run(inputs, trace=False):
    if "nc" not in _CACHE:
        _CACHE["nc"] = _build()
    nc = _CACHE["nc"]
    in_maps = _prep_inputs(**inputs)
    res = run_bass_kernel_spmd(nc, in_maps, list(range(N_CORES)),
                               trace=trace)
    out = np.empty((B, S, HID), np.float32)
    for c in range(N_CORES):
        b, h = c // 2, c % 2
        o = res.results[c]["out"]
        for j in range(4):
            out[b, 512 * j + 256 * h:512 * j + 256 * h + 256, :] = \
                o[256 * j:256 * (j + 1), :]
    return out, res


def kernel(**inputs):
    out, _ = _run(inputs, trace=False)
    return out
